# revision 80
# baseline (speedup 1.0000x reference)
"""Trainium2 Bass kernel for a belief-transformer block (sparse attention).

Computation (per batch b):
    h   = LayerNorm(x[b]) * g1
    qkv = h @ w_qkv ; q,k,v = split(qkv)
    s   = q @ k^T / sqrt(D), keys j >= L_b masked
    y   = softmax(s) @ v
    y   = LayerNorm(y) * g2
    out = gelu(y @ w_fc) @ w_proj

Sharding: data-parallel over batch across 8 NeuronCores (4 slot-batches per
core), weights replicated.  Sparsity: the host sorts batches by
nkc_b = ceil(L_b/128) (number of live 128-key chunks), assigns rank
8s+c to core c slot s, and compiles the program for per-slot chunk
maxima kcs[s] = max over cores.  Key chunks >= kcs[s] are skipped
entirely (k/v production, scores, exp, attn@v); chunks < kcs[s] but
>= the batch's own nkc are masked exactly via the exp bias
(exp(s + log_mask), log_mask in {0, -1e30} per key partition).

Engine layout:
  PE:     transposes (bf16), all matmuls (bf16 operands, fp32 PSUM)
  ACT:    LN applies (Identity w/ scale+bias), exp (with log-mask bias),
          gelu, Y copies; table sets touched only 4x total
          (Sqrt -> Exp -> Sqrt -> Gelu)
  DVE:    bn_stats/aggr for both LNs, psum->SBUF copies, LN2 apply
  Softmax normalization is absorbed into LN2 (scale invariance), with the
  exact eps correction sd = sqrt(var_u + eps*r^2), r = sum_k exp(s).

Phases: 0) LN1 stats for all batches  A) attention per batch (exp set only)
        B) LN2 rstd for all (sqrt set), then apply/transpose/MLP (gelu set).
"""

import os
import sys

import numpy as np

try:
    import ml_dtypes
except ImportError:  # pragma: no cover
    ml_dtypes = None


def _ensure_concourse():
    try:
        import concourse  # noqa: F401
        return
    except ImportError:
        pass
    for p in ("/root/.axon_site/_ro/trn_rl_repo", "/opt/trn_rl_repo"):
        if os.path.isdir(p) and p not in sys.path:
            sys.path.insert(0, p)
    import concourse  # noqa: F401


_ensure_concourse()

import concourse.tile as tile  # noqa: E402
from concourse import bacc, mybir  # noqa: E402
from concourse.bass_utils import run_bass_kernel_spmd  # noqa: E402
from concourse.masks import make_identity  # noqa: E402

B, N, D = 32, 1024, 512
NCORES = 8
G = B // NCORES  # batches (slots) per core
P = 128
NT = N // P      # token chunks (8)
DC = D // P      # feature chunks (4)
HT_ = N // 2     # token half (512)
HC = HT_ // P    # token chunks per half (4)
EPS = 1e-5
NEG = -1e30

F32 = mybir.dt.float32
BF16 = mybir.dt.bfloat16
ALU = mybir.AluOpType
ACT = mybir.ActivationFunctionType


def _body(ctx, tc, x, lb, wdram, out, warm, kcs):
    nc = tc.nc

    singles = ctx.enter_context(tc.tile_pool(name="singles", bufs=1))
    main = ctx.enter_context(tc.tile_pool(name="main", bufs=1))
    vmp = ctx.enter_context(tc.tile_pool(name="vmp", bufs=2))
    work = ctx.enter_context(tc.tile_pool(name="work", bufs=2))
    outp = ctx.enter_context(tc.tile_pool(name="outp", bufs=3))
    stats = ctx.enter_context(tc.tile_pool(name="stats", bufs=2))
    ps_mm = ctx.enter_context(tc.tile_pool(name="ps_mm", bufs=5, space="PSUM"))
    ps_t = ctx.enter_context(tc.tile_pool(name="ps_t", bufs=2, space="PSUM"))
    ps_r = ctx.enter_context(tc.tile_pool(name="ps_r", bufs=1, space="PSUM"))

    # ---- replicated weights, feature-chunked [P, DC, D] bf16 ----
    W = {}

    def load_w(name):
        t = singles.tile([P, DC, D], BF16, tag=name, name=name)
        nc.sync.dma_start(t[:], wdram[name])  # host pre-rearranged layout
        W[name] = t

    ident = singles.tile([P, P], BF16, tag="ident")
    make_identity(nc, ident)
    ones = singles.tile([P, 1], BF16, tag="ones")
    nc.vector.memset(ones[:], 1.0)
    eps_t = singles.tile([P, 1], F32, tag="eps")
    nc.vector.memset(eps_t[:], EPS)


    # persistent per-batch state
    X = singles.tile([P, G, NT, D], BF16, tag="X", name="X")
    LB = singles.tile([P, G, NT], F32, tag="LB", name="LB")
    ZPRES = singles.tile([P, G, 2, DC, HT_], BF16, tag="ZPRES", name="ZPRES")
    MV1 = singles.tile([P, G, NT, 2], F32, tag="MV1", name="MV1")
    RS1 = singles.tile([P, G, NT], F32, tag="RS1", name="RS1")
    NM1 = singles.tile([P, G, NT], F32, tag="NM1", name="NM1")
    MV2 = singles.tile([P, G, 2, HC, 2], F32, tag="MV2", name="MV2")
    RALL = singles.tile([P, G, 2, HC], F32, tag="RALL", name="RALL")
    RSTD2 = singles.tile([P, G, 2, HC], F32, tag="RSTD2", name="RSTD2")
    NM2 = singles.tile([P, G, 2, HC], F32, tag="NM2", name="NM2")

    gate1 = singles.tile([P, 1], F32, tag="gate1", name="gate1")

    S = [dict() for _ in range(G)]  # per-batch transient tiles

    def dma_in(b, split=False):
        # host pre-rearranged x to [P, NT, D]: contiguous 8KB/partition
        if split:
            nc.sync.dma_start(X[:, b, 0:2, :], x[b, :, 0:2, :])
            nc.sync.dma_start(LB[:, b, :], lb[b])
            nc.sync.dma_start(X[:, b, 2:NT, :], x[b, :, 2:NT, :])
        else:
            nc.sync.dma_start(X[:, b, :, :], x[b])
            nc.sync.dma_start(LB[:, b, :], lb[b])

    I32 = mybir.dt.int32

    def newton_rsqrt(dst, vin, w, iters=2):
        """dst[P, w] = 1/sqrt(vin) on DVE (quake seed + Newton iters)."""
        sdsq = stats.tile([P, NT], F32, tag="sdsq", name="sdsq")[:, 0:w]
        nc.vector.tensor_copy(sdsq, vin)
        y = stats.tile([P, NT], F32, tag="nty", name="nty")[:, 0:w]
        nc.vector.tensor_scalar(
            y.bitcast(I32), sdsq.bitcast(I32), 1, -1,
            op0=ALU.logical_shift_right, op1=ALU.bitwise_xor,
        )
        nc.vector.tensor_scalar(
            y.bitcast(I32), y.bitcast(I32), 0x5f3759e0, None, op0=ALU.add,
        )
        for it in range(iters):
            a = stats.tile([P, NT], F32, tag="nta", name="nta")[:, 0:w]
            nc.vector.tensor_tensor(a, y, y, ALU.mult)
            nc.vector.tensor_tensor(a, a, sdsq, ALU.mult)
            nc.vector.tensor_scalar(a, a, -0.5, 1.5, op0=ALU.mult,
                                    op1=ALU.add)
            nc.vector.tensor_tensor(dst if it == iters - 1 else y, y, a,
                                    ALU.mult)

    def ln1_chunk(b, t):
        """LN1 stats for one chunk (DVE only)."""
        st = stats.tile([P, 6], F32, tag="bnst")
        nc.vector.bn_stats(st[:], X[:, b, t, :])
        nc.vector.bn_aggr(MV1[:, b, t, :], st[:])

    def ln1_finish_fast(b, t):
        """Startup-critical rstd1 chain routed through the idle scalar
        engine (Sqrt set loads before the first Exp)."""
        sd = stats.tile([P, 1], F32, tag="sdf")
        nc.scalar.activation(sd[:], MV1[:, b, t, 1:2], ACT.Sqrt,
                             bias=eps_t[:])
        nc.vector.reciprocal(RS1[:, b, t:t + 1], sd[:])
        nc.vector.tensor_scalar(
            NM1[:, b, t:t + 1], MV1[:, b, t, 0:1], RS1[:, b, t:t + 1], -1.0,
            op0=ALU.mult, op1=ALU.mult,
        )

    def ln1_finish(b, t0, t1):
        """rstd1/negmu1 for chunks [t0,t1) via DVE Newton."""
        w = t1 - t0
        veps = stats.tile([P, NT], F32, tag="veps", name="veps")[:, 0:w]
        nc.vector.tensor_scalar(
            veps, MV1[:, b, t0:t1, 1:2], 1.0, EPS, op0=ALU.mult, op1=ALU.add
        )
        newton_rsqrt(RS1[:, b, t0:t1], veps, w, iters=1)
        nm = stats.tile([P, NT], F32, tag="nm1t", name="nm1t")[:, 0:w]
        nc.vector.tensor_tensor(
            nm, MV1[:, b, t0:t1, 0:1], RS1[:, b, t0:t1], ALU.mult
        )
        nc.vector.tensor_scalar(
            NM1[:, b, t0:t1], nm, -1.0, None, op0=ALU.mult
        )

    def emit_gate1():
        """gate1 = 1.0 once the last batch's exp outputs (PT tiles) are
        written — the earliest moment the gelu table switch is legal."""
        s = S[G - 1]
        kc = kcs[G - 1]
        tmp1 = stats.tile([P, 1], F32, tag="gtmp")
        nc.vector.tensor_reduce(tmp1[:], s["PT0"][:, 0:kc, 0:1],
                                axis=mybir.AxisListType.XY, op=ALU.max)
        tmp2 = stats.tile([P, 1], F32, tag="gtmp2")
        nc.vector.tensor_reduce(tmp2[:], s["PT1"][:, 0:kc, 0:1],
                                axis=mybir.AxisListType.XY, op=ALU.max)
        nc.vector.tensor_tensor(tmp1[:], tmp1[:], tmp2[:], ALU.max)
        nc.vector.tensor_scalar(
            gate1[:], tmp1[:], 0.0, 1.0, op0=ALU.mult, op1=ALU.add
        )

    def emit_H_chunk(b, t, on_scalar=False):
        """LN1 apply: H = rstd*x + (-mu*rstd), bf16 in/out."""
        s = S[b]
        if on_scalar:
            nc.scalar.activation(
                s["H"][:, t, :], X[:, b, t, :], ACT.Identity,
                bias=NM1[:, b, t:t + 1], scale=RS1[:, b, t:t + 1],
            )
        else:
            nc.vector.tensor_scalar(
                s["H"][:, t, :], X[:, b, t, :], RS1[:, b, t:t + 1],
                NM1[:, b, t:t + 1], op0=ALU.mult, op1=ALU.add,
            )

    def emit_B_alloc(b):
        s = S[b]
        s["H"] = main.tile([P, NT, D], BF16, tag="H", name="H")
        s["HT"] = main.tile([P, DC, N], BF16, tag="HT", name="HT")
        s["VM"] = vmp.tile([P, NT, D], BF16, tag="VM", name="VM")

    def emit_Bt(b, t, kc):
        """Transpose h chunk t -> h_T; v-matmul for live chunks."""
        s = S[b]
        pt = ps_t.tile([P, DC, P], BF16, tag="pst")
        for c in range(DC):
            nc.tensor.transpose(
                pt[:, c, :], s["H"][:, t, c * P:(c + 1) * P], ident[:]
            )
        nc.vector.tensor_copy(s["HT"][:, :, t * P:(t + 1) * P], pt[:])
        if t < kc:
            pm = ps_mm.tile([P, 512], F32, tag="psmm")
            for dc_ in range(DC):
                nc.tensor.matmul(
                    pm[:],
                    s["HT"][:, dc_, t * P:(t + 1) * P],
                    W["wv"][:, dc_, :],
                    start=(dc_ == 0), stop=(dc_ == DC - 1),
                )
            nc.scalar.activation(s["VM"][:, t, :], pm[:], ACT.Identity)

    def emit_C_alloc(b):
        s = S[b]
        s["QT"] = main.tile([P, DC, N], BF16, tag="QT", name="QT")
        s["KT"] = main.tile([P, DC, N], BF16, tag="KT", name="KT")

    def emit_Q_piece(b, h, c):
        """One PSUM group of q_T production (4 matmuls, 512 tokens)."""
        s = S[b]
        pm = ps_mm.tile([P, 512], F32, tag="psmm")
        for dc_ in range(DC):
            nc.tensor.matmul(
                pm[:],
                W["wq"][:, dc_, c * P:(c + 1) * P],
                s["HT"][:, dc_, h * 512:(h + 1) * 512],
                start=(dc_ == 0), stop=(dc_ == DC - 1),
            )
        nc.scalar.activation(
            s["QT"][:, c, h * 512:(h + 1) * 512], pm[:], ACT.Identity
        )

    def emit_K_span(b, h, c):
        """k_T production for a full 512-key span."""
        s = S[b]
        pm = ps_mm.tile([P, 512], F32, tag="psmm")
        for dc_ in range(DC):
            nc.tensor.matmul(
                pm[:],
                W["wk"][:, dc_, c * P:(c + 1) * P],
                s["HT"][:, dc_, h * 512:(h + 1) * 512],
                start=(dc_ == 0), stop=(dc_ == DC - 1),
            )
        nc.vector.tensor_copy(s["KT"][:, c, h * 512:(h + 1) * 512], pm[:])

    def emit_K_chunk(b, jc, c):
        """k_T production for one 128-key chunk (remainder)."""
        s = S[b]
        pk = ps_mm.tile([P, 512], F32, tag="psmm", name="psk")
        for dc_ in range(DC):
            nc.tensor.matmul(
                pk[:, 0:P],
                W["wk"][:, dc_, c * P:(c + 1) * P],
                s["HT"][:, dc_, jc * P:(jc + 1) * P],
                start=(dc_ == 0), stop=(dc_ == DC - 1),
            )
        nc.vector.tensor_copy(s["KT"][:, c, jc * P:(jc + 1) * P], pk[:, 0:P])

    def emit_front(b, kc, with_stats=False):
        """H, transposes+v, q, k for batch b, pipelined per token chunk."""
        emit_B_alloc(b)
        emit_C_alloc(b)
        nspan = kc // 4

        def chunk_work(t, h_on_scalar=False):
            emit_H_chunk(b, t, on_scalar=h_on_scalar)
            emit_Bt(b, t, kc)
            if t >= 4 * nspan and t < kc:
                for c in range(DC):
                    emit_K_chunk(b, t, c)
            if t == 3:
                for c in range(DC):
                    emit_Q_piece(b, 0, c)
                if nspan >= 1:
                    for c in range(DC):
                        emit_K_span(b, 0, c)
            if t == 7:
                for c in range(DC):
                    emit_Q_piece(b, 1, c)
                if nspan >= 2:
                    for c in range(DC):
                        emit_K_span(b, 1, c)

        if with_stats:
            # first two chunks: rstd chain via the idle scalar engine so
            # the DVE bn backlog can't stretch the critical path
            for t in (0, 1):
                ln1_chunk(b, t)
                ln1_finish_fast(b, t)
                chunk_work(t, h_on_scalar=True)
            for tp in range(1, NT // 2):
                ln1_chunk(b, 2 * tp)
                ln1_chunk(b, 2 * tp + 1)
                ln1_finish(b, 2 * tp, 2 * tp + 2)
                chunk_work(2 * tp)
                chunk_work(2 * tp + 1)
        else:
            for t in range(NT):
                chunk_work(t)

    def emit_D_alloc(b, hf):
        S[b][f"PT{hf}"] = main.tile([P, NT, HT_], BF16, tag=f"PT{hf}",
                                    name="PT")

    def emit_D(b, hf, jc):
        """Scores for key-chunk jc (keys on partitions) + masked exp."""
        s = S[b]
        q0 = hf * HT_
        pm = ps_mm.tile([P, 512], F32, tag="psmm")
        for dc_ in range(DC):
            nc.tensor.matmul(
                pm[:],
                s["KT"][:, dc_, jc * P:(jc + 1) * P],
                s["QT"][:, dc_, q0:q0 + HT_],
                start=(dc_ == 0), stop=(dc_ == DC - 1),
            )
        nc.scalar.activation(
            s[f"PT{hf}"][:, jc, :], pm[:], ACT.Exp, bias=LB[:, b, jc:jc + 1]
        )

    def emit_E(b, hf, kc):
        """y_unnorm = p^T @ v; rowsums r via 1-col matmuls; LN2 stats."""
        s = S[b]
        PT = s[f"PT{hf}"]
        s[f"Y{hf}"] = work.tile([P, HC, D], BF16, tag=f"Y{hf}", name="Y")
        for il in range(HC):
            pm = ps_mm.tile([P, 512], F32, tag="psmm")
            pr = ps_r.tile([P, 1], F32, tag="psr", name="pr")
            for jc in range(kc):
                nc.tensor.matmul(
                    pm[:],
                    PT[:, jc, il * P:(il + 1) * P],
                    s["VM"][:, jc, :],
                    start=(jc == 0), stop=(jc == kc - 1),
                )
                nc.tensor.matmul(
                    pr[:],
                    PT[:, jc, il * P:(il + 1) * P],
                    ones[:],
                    start=(jc == 0), stop=(jc == kc - 1),
                )
            st = stats.tile([P, 6], F32, tag="bnst")
            nc.vector.bn_stats(st[:], pm[:])
            nc.vector.bn_aggr(MV2[:, b, hf, il, :], st[:])
            nc.scalar.activation(s[f"Y{hf}"][:, il, :], pm[:], ACT.Identity)
            nc.vector.tensor_copy(RALL[:, b, hf, il:il + 1], pr[:])

    def emit_rstd2(b, hf):
        """rstd2 = 1/sqrt(var_u + eps*r^2) via DVE Newton (no scalar sqrt,
        so this runs inside phase A without touching activation tables)."""
        r = RALL[:, b, hf, :]
        sdsq = stats.tile([P, HC], F32, tag="sdsq2")
        nc.vector.scalar_tensor_tensor(
            sdsq[:], r, EPS, r, op0=ALU.mult, op1=ALU.mult
        )
        nc.vector.tensor_tensor(
            sdsq[:], sdsq[:], MV2[:, b, hf, :, 1:2], ALU.add
        )
        newton_rsqrt(RSTD2[:, b, hf, :], sdsq[:], HC)
        nm = stats.tile([P, HC], F32, tag="nm2t")
        nc.vector.tensor_tensor(
            nm[:], MV2[:, b, hf, :, 0:1], RSTD2[:, b, hf, :], ALU.mult
        )
        nc.vector.tensor_scalar(
            NM2[:, b, hf, :], nm[:], -1.0, None, op0=ALU.mult
        )

    def emit_tail_head(b, hf, direct=False):
        """LN2 apply, transpose, fc matmuls -> ZPRES (runs in phase A).
        With direct=True (only legal after gate1), gelu straight from the
        fc psum, skipping the ZPRES round-trip."""
        s = S[b]
        YB = work.tile([P, HC, D], BF16, tag="YB", name="YB")
        for il in range(HC):
            nc.vector.tensor_scalar(
                YB[:, il, :], s[f"Y{hf}"][:, il, :],
                RSTD2[:, b, hf, il:il + 1], NM2[:, b, hf, il:il + 1],
                op0=ALU.mult, op1=ALU.add,
            )
        YLT = work.tile([P, DC, HT_], BF16, tag="YLT", name="YLT")
        for tl in range(HC):
            pt = ps_t.tile([P, DC, P], BF16, tag="pst")
            for c in range(DC):
                nc.tensor.transpose(
                    pt[:, c, :], YB[:, tl, c * P:(c + 1) * P], ident[:]
                )
            nc.vector.tensor_copy(YLT[:, :, tl * P:(tl + 1) * P], pt[:])
        if direct:
            ZT = work.tile([P, DC, HT_], BF16, tag="ZTD", name="ZTD")
            s[f"ZT{hf}"] = ZT
        for c in range(DC):
            pm = ps_mm.tile([P, 512], F32, tag="psmm")
            for ec in range(DC):
                nc.tensor.matmul(
                    pm[:],
                    W["wf"][:, ec, c * P:(c + 1) * P],
                    YLT[:, ec, :],
                    start=(ec == 0), stop=(ec == DC - 1),
                )
            if direct:
                nc.scalar.activation(ZT[:, c, :], pm[:], ACT.Gelu,
                                     scale=gate1[:])
            else:
                nc.scalar.activation(ZPRES[:, b, hf, c, :], pm[:],
                                     ACT.Identity)

    def emit_tail_fin(b, hf, direct=False):
        """gelu (gated to the final gelu-set region), proj, store."""
        if direct:
            ZT = S[b][f"ZT{hf}"]
        else:
            ZT = work.tile([P, DC, HT_], BF16, tag="ZT", name="ZT")
            nc.scalar.activation(
                ZT[:, :, :], ZPRES[:, b, hf, :, :], ACT.Gelu, scale=gate1[:]
            )
        ob = out[b]
        for il in range(HC):
            pm = ps_mm.tile([P, 512], F32, tag="psmm")
            for c in range(DC):
                nc.tensor.matmul(
                    pm[:],
                    ZT[:, c, il * P:(il + 1) * P],
                    W["wp"][:, c, :],
                    start=(c == 0), stop=(c == DC - 1),
                )
            o = outp.tile([P, D], F32, tag="O")
            nc.vector.tensor_copy(o[:], pm[:])
            nc.sync.dma_start(ob[:, hf * HC + il, :], o[:])

    # ---------------- emission ----------------
    # startup DMA order: wv (PE warm-up feed), x[0] (LN1 critical path),
    # then weights interleaved with the remaining batches.
    load_w("wv")
    dma_in(0, split=True)
    load_w("wq")
    dma_in(1)
    load_w("wk")
    dma_in(2)
    load_w("wf")
    dma_in(3)
    load_w("wp")

    def warm_ident(n_mm):
        """HAM warm-up on the identity tile — no DMA dependency."""
        wpm = ps_mm.tile([P, 512], F32, tag="psmm", name="warmmm")
        for k in range(n_mm):
            nc.tensor.matmul(
                wpm[:, 0:P], ident[:], ident[:],
                start=(k == 0), stop=(k == n_mm - 1),
            )

    def warm_burst(k0, n_mm, last):
        wpm = ps_mm.tile([P, 512], F32, tag="psmm", name="warmmm")
        for k in range(n_mm):
            nc.tensor.matmul(
                wpm[:], W["wv"][:, (k0 + k) % DC, 0:P],
                W["wv"][:, (k0 + k) % DC, :],
                start=(k == 0), stop=(k == n_mm - 1),
            )
        if last:
            wsb = outp.tile([P, 8], F32, tag="O", name="warmsb")
            nc.vector.tensor_copy(wsb[:], wpm[:, 0:8])
            nc.sync.dma_start(warm[:], wsb[:])

    warm_ident(30)
    warm_burst(0, 20, False)

    # batch 0 front with fused LN1 stats; stats for the rest (DVE only)
    emit_front(0, kcs[0], with_stats=True)
    warm_burst(16, 8, True)
    for b in range(1, G):
        for t in range(NT):
            ln1_chunk(b, t)
        ln1_finish(b, 0, NT)

    # phase A; each tail_head is delayed half an iteration so its PE work
    # (transposes + fc) never head-of-line-blocks on the LN2 DVE chain.
    # Once the last batch's exps are emitted (end of its D phase), gate1
    # opens and earlier batches' gelu+proj fins are interleaved as PE
    # filler for the final batch's serial E/LN2 chain.
    for b in range(G):
        kc = kcs[b]
        emit_D_alloc(b, 0)
        emit_D_alloc(b, 1)
        for jc in range(kc):
            emit_D(b, 0, jc)
        for jc in range(kc):
            emit_D(b, 1, jc)
        if b >= 1:
            emit_tail_head(b - 1, 1)
        if b == G - 1:
            emit_gate1()
            emit_tail_fin(0, 0)
        emit_E(b, 0, kc)
        emit_rstd2(b, 0)
        if b + 1 < G:
            emit_front(b + 1, kcs[b + 1])
        else:
            emit_tail_fin(0, 1)
            emit_tail_fin(1, 0)
        emit_tail_head(b, 0, direct=(b == G - 1))
        if b == G - 1:
            emit_tail_fin(1, 1)
        emit_E(b, 1, kc)
        emit_rstd2(b, 1)
    emit_tail_fin(2, 0)
    emit_tail_fin(2, 1)
    emit_tail_fin(G - 1, 0, direct=True)
    emit_tail_head(G - 1, 1, direct=True)
    emit_tail_fin(G - 1, 1, direct=True)


def build(kcs):
    from contextlib import ExitStack

    nc = bacc.Bacc("TRN2", target_bir_lowering=False, debug=False,
                   num_devices=NCORES)
    # x/lb/out are pre-rearranged on the host to the on-chip chunk layout
    # [P, NT, ...] (token n = t*P + p) so every DMA line is contiguous.
    x = nc.dram_tensor("x", [G, P, NT, D], BF16, kind="ExternalInput").ap()
    lb = nc.dram_tensor("lb", [G, P, NT], F32, kind="ExternalInput").ap()
    wdram = {
        name: nc.dram_tensor(name, [P, DC, D], BF16, kind="ExternalInput").ap()
        for name in ("wq", "wk", "wv", "wf", "wp")
    }
    out = nc.dram_tensor("out", [G, P, NT, D], F32, kind="ExternalOutput").ap()
    warm = nc.dram_tensor("warm", [P, 8], F32, kind="ExternalOutput").ap()

    with tile.TileContext(nc) as tc:
        with ExitStack() as ctx:
            _body(ctx, tc, x, lb, wdram, out, warm, kcs)
    nc.compile()
    return nc


_NC_CACHE = {}


def get_nc(kcs):
    kcs = tuple(kcs)
    if kcs not in _NC_CACHE:
        _NC_CACHE[kcs] = build(kcs)
    return _NC_CACHE[kcs]


def compute_schedule(belief_base_sizes):
    """Sort batches by live-chunk count desc; slot s takes ranks [8s, 8s+8).

    Returns (order, kcs): order[s*8+c] = original batch index run on core c
    slot s; kcs[s] = max chunk count in slot s (compiled loop bound).
    """
    sizes = np.asarray(belief_base_sizes).astype(np.int64)
    nkc = (sizes + P - 1) // P
    nkc = np.clip(nkc, 1, NT)
    order = np.argsort(-nkc, kind="stable")
    kcs = tuple(int(nkc[order[s * NCORES]]) for s in range(G))
    return order, kcs


def make_in_maps(x, belief_base_sizes, g1, w_qkv, g2, w_fc, w_proj):
    x = np.asarray(x, dtype=np.float32)
    sizes = np.asarray(belief_base_sizes, dtype=np.int64)
    g1 = np.asarray(g1, dtype=np.float32)
    w_qkv = np.asarray(w_qkv, dtype=np.float32)
    g2 = np.asarray(g2, dtype=np.float32)
    w_fc = np.asarray(w_fc, dtype=np.float32)
    w_proj = np.asarray(w_proj, dtype=np.float32)

    bf = ml_dtypes.bfloat16

    def warr(w):
        # device layout [P, DC, D]: partition p holds rows {c*P+p}
        return np.ascontiguousarray(
            w.astype(bf).reshape(DC, P, D).transpose(1, 0, 2))

    wq = warr((g1[:, None] * w_qkv[:, :D]) / np.float32(np.sqrt(D)))
    wk = warr(g1[:, None] * w_qkv[:, D:2 * D])
    wv = warr(g1[:, None] * w_qkv[:, 2 * D:])
    wf = warr(g2[:, None] * w_fc)
    wp = warr(w_proj)

    lbias = np.where(np.arange(N)[None, :] < sizes[:, None],
                     np.float32(0.0), np.float32(NEG)).astype(np.float32)

    order, kcs = compute_schedule(sizes)
    # rearrange to the on-chip chunk layout [P, NT, .] (token n = t*P + p)
    xb = x.astype(bf).reshape(B, NT, P, D).transpose(0, 2, 1, 3)
    lbias = lbias.reshape(B, NT, P).transpose(0, 2, 1)
    in_maps = []
    for c in range(NCORES):
        sel = [int(order[s * NCORES + c]) for s in range(G)]
        in_maps.append({
            "x": np.ascontiguousarray(xb[sel]),
            "lb": np.ascontiguousarray(lbias[sel]),
            "wq": np.ascontiguousarray(wq), "wk": np.ascontiguousarray(wk),
            "wv": np.ascontiguousarray(wv), "wf": np.ascontiguousarray(wf),
            "wp": np.ascontiguousarray(wp),
        })
    return in_maps, order, kcs


def kernel(x, belief_base_sizes, g1, w_qkv, g2, w_fc, w_proj):
    in_maps, order, kcs = make_in_maps(
        x, belief_base_sizes, g1, w_qkv, g2, w_fc, w_proj)
    nc = get_nc(kcs)
    res = run_bass_kernel_spmd(nc, in_maps, core_ids=list(range(NCORES)))
    out = np.empty((B, N, D), dtype=np.float32)
    for c in range(NCORES):
        for s in range(G):
            buf = res.results[c]["out"][s]  # [P, NT, D], token n = t*P + p
            out[int(order[s * NCORES + c])] = (
                buf.transpose(1, 0, 2).reshape(N, D))
    return np.ascontiguousarray(out)


# revision 81
# speedup vs baseline: 1.0024x; 1.0024x over previous
"""Trainium2 Bass kernel for a belief-transformer block (sparse attention).

Computation (per batch b):
    h   = LayerNorm(x[b]) * g1
    qkv = h @ w_qkv ; q,k,v = split(qkv)
    s   = q @ k^T / sqrt(D), keys j >= L_b masked
    y   = softmax(s) @ v
    y   = LayerNorm(y) * g2
    out = gelu(y @ w_fc) @ w_proj

Sharding: data-parallel over batch across 8 NeuronCores (4 slot-batches per
core), weights replicated.  Sparsity: the host sorts batches by
nkc_b = ceil(L_b/128) (number of live 128-key chunks), assigns rank
8s+c to core c slot s, and compiles the program for per-slot chunk
maxima kcs[s] = max over cores.  Key chunks >= kcs[s] are skipped
entirely (k/v production, scores, exp, attn@v); chunks < kcs[s] but
>= the batch's own nkc are masked exactly via the exp bias
(exp(s + log_mask), log_mask in {0, -1e30} per key partition).

Engine layout:
  PE:     transposes (bf16), all matmuls (bf16 operands, fp32 PSUM)
  ACT:    LN applies (Identity w/ scale+bias), exp (with log-mask bias),
          gelu, Y copies; table sets touched only 4x total
          (Sqrt -> Exp -> Sqrt -> Gelu)
  DVE:    bn_stats/aggr for both LNs, psum->SBUF copies, LN2 apply
  Softmax normalization is absorbed into LN2 (scale invariance), with the
  exact eps correction sd = sqrt(var_u + eps*r^2), r = sum_k exp(s).

Phases: 0) LN1 stats for all batches  A) attention per batch (exp set only)
        B) LN2 rstd for all (sqrt set), then apply/transpose/MLP (gelu set).
"""

import os
import sys

import numpy as np

try:
    import ml_dtypes
except ImportError:  # pragma: no cover
    ml_dtypes = None


def _ensure_concourse():
    try:
        import concourse  # noqa: F401
        return
    except ImportError:
        pass
    for p in ("/root/.axon_site/_ro/trn_rl_repo", "/opt/trn_rl_repo"):
        if os.path.isdir(p) and p not in sys.path:
            sys.path.insert(0, p)
    import concourse  # noqa: F401


_ensure_concourse()

import concourse.tile as tile  # noqa: E402
from concourse import bacc, mybir  # noqa: E402
from concourse.bass_utils import run_bass_kernel_spmd  # noqa: E402
from concourse.masks import make_identity  # noqa: E402

B, N, D = 32, 1024, 512
NCORES = 8
G = B // NCORES  # batches (slots) per core
P = 128
NT = N // P      # token chunks (8)
DC = D // P      # feature chunks (4)
HT_ = N // 2     # token half (512)
HC = HT_ // P    # token chunks per half (4)
EPS = 1e-5
NEG = -1e30

F32 = mybir.dt.float32
BF16 = mybir.dt.bfloat16
ALU = mybir.AluOpType
ACT = mybir.ActivationFunctionType


def _body(ctx, tc, x, lb, wdram, out, warm, kcs):
    nc = tc.nc

    singles = ctx.enter_context(tc.tile_pool(name="singles", bufs=1))
    main = ctx.enter_context(tc.tile_pool(name="main", bufs=1))
    vmp = ctx.enter_context(tc.tile_pool(name="vmp", bufs=2))
    work = ctx.enter_context(tc.tile_pool(name="work", bufs=2))
    outp = ctx.enter_context(tc.tile_pool(name="outp", bufs=3))
    stats = ctx.enter_context(tc.tile_pool(name="stats", bufs=2))
    ps_mm = ctx.enter_context(tc.tile_pool(name="ps_mm", bufs=4, space="PSUM"))
    ps_t = ctx.enter_context(tc.tile_pool(name="ps_t", bufs=2, space="PSUM"))
    ps_r = ctx.enter_context(tc.tile_pool(name="ps_r", bufs=2, space="PSUM"))

    # ---- replicated weights, feature-chunked [P, DC, D] bf16 ----
    W = {}

    def load_w(name):
        t = singles.tile([P, DC, D], BF16, tag=name, name=name)
        nc.sync.dma_start(t[:], wdram[name])  # host pre-rearranged layout
        W[name] = t

    ident = singles.tile([P, P], BF16, tag="ident")
    make_identity(nc, ident)
    ones = singles.tile([P, 1], BF16, tag="ones")
    nc.vector.memset(ones[:], 1.0)
    eps_t = singles.tile([P, 1], F32, tag="eps")
    nc.vector.memset(eps_t[:], EPS)


    # persistent per-batch state
    X = singles.tile([P, G, NT, D], BF16, tag="X", name="X")
    LB = singles.tile([P, G, NT], F32, tag="LB", name="LB")
    ZPRES = singles.tile([P, G, 2, DC, HT_], BF16, tag="ZPRES", name="ZPRES")
    MV1 = singles.tile([P, G, NT, 2], F32, tag="MV1", name="MV1")
    RS1 = singles.tile([P, G, NT], F32, tag="RS1", name="RS1")
    NM1 = singles.tile([P, G, NT], F32, tag="NM1", name="NM1")
    MV2 = singles.tile([P, G, 2, HC, 2], F32, tag="MV2", name="MV2")
    RALL = singles.tile([P, G, 2, HC], F32, tag="RALL", name="RALL")
    RSTD2 = singles.tile([P, G, 2, HC], F32, tag="RSTD2", name="RSTD2")
    NM2 = singles.tile([P, G, 2, HC], F32, tag="NM2", name="NM2")

    gate1 = singles.tile([P, 1], F32, tag="gate1", name="gate1")

    S = [dict() for _ in range(G)]  # per-batch transient tiles

    def dma_in(b, split=False):
        # host pre-rearranged x to [P, NT, D]: contiguous 8KB/partition
        if split:
            nc.sync.dma_start(X[:, b, 0:2, :], x[b, :, 0:2, :])
            nc.sync.dma_start(LB[:, b, :], lb[b])
            nc.sync.dma_start(X[:, b, 2:NT, :], x[b, :, 2:NT, :])
        else:
            nc.sync.dma_start(X[:, b, :, :], x[b])
            nc.sync.dma_start(LB[:, b, :], lb[b])

    I32 = mybir.dt.int32

    def newton_rsqrt(dst, vin, w, iters=2):
        """dst[P, w] = 1/sqrt(vin) on DVE (quake seed + Newton iters)."""
        sdsq = stats.tile([P, NT], F32, tag="sdsq", name="sdsq")[:, 0:w]
        nc.vector.tensor_copy(sdsq, vin)
        y = stats.tile([P, NT], F32, tag="nty", name="nty")[:, 0:w]
        nc.vector.tensor_scalar(
            y.bitcast(I32), sdsq.bitcast(I32), 1, -1,
            op0=ALU.logical_shift_right, op1=ALU.bitwise_xor,
        )
        nc.vector.tensor_scalar(
            y.bitcast(I32), y.bitcast(I32), 0x5f3759e0, None, op0=ALU.add,
        )
        for it in range(iters):
            a = stats.tile([P, NT], F32, tag="nta", name="nta")[:, 0:w]
            nc.vector.tensor_tensor(a, y, y, ALU.mult)
            nc.vector.tensor_tensor(a, a, sdsq, ALU.mult)
            nc.vector.tensor_scalar(a, a, -0.5, 1.5, op0=ALU.mult,
                                    op1=ALU.add)
            nc.vector.tensor_tensor(dst if it == iters - 1 else y, y, a,
                                    ALU.mult)

    def ln1_chunk(b, t):
        """LN1 stats for one chunk (DVE only)."""
        st = stats.tile([P, 6], F32, tag="bnst")
        nc.vector.bn_stats(st[:], X[:, b, t, :])
        nc.vector.bn_aggr(MV1[:, b, t, :], st[:])

    def ln1_finish_fast(b, t):
        """Startup-critical rstd1 chain routed through the idle scalar
        engine (Sqrt set loads before the first Exp)."""
        sd = stats.tile([P, 1], F32, tag="sdf")
        nc.scalar.activation(sd[:], MV1[:, b, t, 1:2], ACT.Sqrt,
                             bias=eps_t[:])
        nc.vector.reciprocal(RS1[:, b, t:t + 1], sd[:])
        nc.vector.tensor_scalar(
            NM1[:, b, t:t + 1], MV1[:, b, t, 0:1], RS1[:, b, t:t + 1], -1.0,
            op0=ALU.mult, op1=ALU.mult,
        )

    def ln1_finish(b, t0, t1):
        """rstd1/negmu1 for chunks [t0,t1) via DVE Newton."""
        w = t1 - t0
        veps = stats.tile([P, NT], F32, tag="veps", name="veps")[:, 0:w]
        nc.vector.tensor_scalar(
            veps, MV1[:, b, t0:t1, 1:2], 1.0, EPS, op0=ALU.mult, op1=ALU.add
        )
        newton_rsqrt(RS1[:, b, t0:t1], veps, w, iters=1)
        nm = stats.tile([P, NT], F32, tag="nm1t", name="nm1t")[:, 0:w]
        nc.vector.tensor_tensor(
            nm, MV1[:, b, t0:t1, 0:1], RS1[:, b, t0:t1], ALU.mult
        )
        nc.vector.tensor_scalar(
            NM1[:, b, t0:t1], nm, -1.0, None, op0=ALU.mult
        )

    def emit_gate1():
        """gate1 = 1.0 once the last batch's exp outputs (PT tiles) are
        written — the earliest moment the gelu table switch is legal."""
        s = S[G - 1]
        kc = kcs[G - 1]
        tmp1 = stats.tile([P, 1], F32, tag="gtmp")
        nc.vector.tensor_reduce(tmp1[:], s["PT0"][:, 0:kc, 0:1],
                                axis=mybir.AxisListType.XY, op=ALU.max)
        tmp2 = stats.tile([P, 1], F32, tag="gtmp2")
        nc.vector.tensor_reduce(tmp2[:], s["PT1"][:, 0:kc, 0:1],
                                axis=mybir.AxisListType.XY, op=ALU.max)
        nc.vector.tensor_tensor(tmp1[:], tmp1[:], tmp2[:], ALU.max)
        nc.vector.tensor_scalar(
            gate1[:], tmp1[:], 0.0, 1.0, op0=ALU.mult, op1=ALU.add
        )

    def emit_H_chunk(b, t, on_scalar=False):
        """LN1 apply: H = rstd*x + (-mu*rstd), bf16 in/out."""
        s = S[b]
        if on_scalar:
            nc.scalar.activation(
                s["H"][:, t, :], X[:, b, t, :], ACT.Identity,
                bias=NM1[:, b, t:t + 1], scale=RS1[:, b, t:t + 1],
            )
        else:
            nc.vector.tensor_scalar(
                s["H"][:, t, :], X[:, b, t, :], RS1[:, b, t:t + 1],
                NM1[:, b, t:t + 1], op0=ALU.mult, op1=ALU.add,
            )

    def emit_B_alloc(b):
        s = S[b]
        s["H"] = main.tile([P, NT, D], BF16, tag="H", name="H")
        s["HT"] = main.tile([P, DC, N], BF16, tag="HT", name="HT")
        s["VM"] = vmp.tile([P, NT, D], BF16, tag="VM", name="VM")

    def emit_Bt(b, t, kc):
        """Transpose h chunk t -> h_T; v-matmul for live chunks."""
        s = S[b]
        pt = ps_t.tile([P, DC, P], BF16, tag="pst")
        for c in range(DC):
            nc.tensor.transpose(
                pt[:, c, :], s["H"][:, t, c * P:(c + 1) * P], ident[:]
            )
        nc.vector.tensor_copy(s["HT"][:, :, t * P:(t + 1) * P], pt[:])
        if t < kc:
            pm = ps_mm.tile([P, 512], F32, tag="psmm")
            for dc_ in range(DC):
                nc.tensor.matmul(
                    pm[:],
                    s["HT"][:, dc_, t * P:(t + 1) * P],
                    W["wv"][:, dc_, :],
                    start=(dc_ == 0), stop=(dc_ == DC - 1),
                )
            nc.scalar.activation(s["VM"][:, t, :], pm[:], ACT.Identity)

    def emit_C_alloc(b):
        s = S[b]
        s["QT"] = main.tile([P, DC, N], BF16, tag="QT", name="QT")
        s["KT"] = main.tile([P, DC, N], BF16, tag="KT", name="KT")

    def emit_Q_piece(b, h, c):
        """One PSUM group of q_T production (4 matmuls, 512 tokens)."""
        s = S[b]
        pm = ps_mm.tile([P, 512], F32, tag="psmm")
        for dc_ in range(DC):
            nc.tensor.matmul(
                pm[:],
                W["wq"][:, dc_, c * P:(c + 1) * P],
                s["HT"][:, dc_, h * 512:(h + 1) * 512],
                start=(dc_ == 0), stop=(dc_ == DC - 1),
            )
        nc.scalar.activation(
            s["QT"][:, c, h * 512:(h + 1) * 512], pm[:], ACT.Identity
        )

    def emit_K_span(b, h, c):
        """k_T production for a full 512-key span."""
        s = S[b]
        pm = ps_mm.tile([P, 512], F32, tag="psmm")
        for dc_ in range(DC):
            nc.tensor.matmul(
                pm[:],
                W["wk"][:, dc_, c * P:(c + 1) * P],
                s["HT"][:, dc_, h * 512:(h + 1) * 512],
                start=(dc_ == 0), stop=(dc_ == DC - 1),
            )
        nc.vector.tensor_copy(s["KT"][:, c, h * 512:(h + 1) * 512], pm[:])

    def emit_K_chunk(b, jc, c):
        """k_T production for one 128-key chunk (remainder)."""
        s = S[b]
        pk = ps_mm.tile([P, 512], F32, tag="psmm", name="psk")
        for dc_ in range(DC):
            nc.tensor.matmul(
                pk[:, 0:P],
                W["wk"][:, dc_, c * P:(c + 1) * P],
                s["HT"][:, dc_, jc * P:(jc + 1) * P],
                start=(dc_ == 0), stop=(dc_ == DC - 1),
            )
        nc.vector.tensor_copy(s["KT"][:, c, jc * P:(jc + 1) * P], pk[:, 0:P])

    def emit_front(b, kc, with_stats=False):
        """H, transposes+v, q, k for batch b, pipelined per token chunk."""
        emit_B_alloc(b)
        emit_C_alloc(b)
        nspan = kc // 4

        def chunk_work(t, h_on_scalar=False):
            emit_H_chunk(b, t, on_scalar=h_on_scalar)
            emit_Bt(b, t, kc)
            if t >= 4 * nspan and t < kc:
                for c in range(DC):
                    emit_K_chunk(b, t, c)
            if t == 3:
                for c in range(DC):
                    emit_Q_piece(b, 0, c)
                if nspan >= 1:
                    for c in range(DC):
                        emit_K_span(b, 0, c)
            if t == 7:
                for c in range(DC):
                    emit_Q_piece(b, 1, c)
                if nspan >= 2:
                    for c in range(DC):
                        emit_K_span(b, 1, c)

        if with_stats:
            # first two chunks: rstd chain via the idle scalar engine so
            # the DVE bn backlog can't stretch the critical path
            for t in (0, 1):
                ln1_chunk(b, t)
                ln1_finish_fast(b, t)
                chunk_work(t, h_on_scalar=True)
            for tp in range(1, NT // 2):
                ln1_chunk(b, 2 * tp)
                ln1_chunk(b, 2 * tp + 1)
                ln1_finish(b, 2 * tp, 2 * tp + 2)
                chunk_work(2 * tp)
                chunk_work(2 * tp + 1)
        else:
            for t in range(NT):
                chunk_work(t)

    def emit_D_alloc(b, hf):
        S[b][f"PT{hf}"] = main.tile([P, NT, HT_], BF16, tag=f"PT{hf}",
                                    name="PT")

    def emit_D(b, hf, jc):
        """Scores for key-chunk jc (keys on partitions) + masked exp."""
        s = S[b]
        q0 = hf * HT_
        pm = ps_mm.tile([P, 512], F32, tag="psmm")
        for dc_ in range(DC):
            nc.tensor.matmul(
                pm[:],
                s["KT"][:, dc_, jc * P:(jc + 1) * P],
                s["QT"][:, dc_, q0:q0 + HT_],
                start=(dc_ == 0), stop=(dc_ == DC - 1),
            )
        nc.scalar.activation(
            s[f"PT{hf}"][:, jc, :], pm[:], ACT.Exp, bias=LB[:, b, jc:jc + 1]
        )

    def emit_E(b, hf, kc):
        """y_unnorm = p^T @ v; rowsums r via 1-col matmuls; LN2 stats."""
        s = S[b]
        PT = s[f"PT{hf}"]
        s[f"Y{hf}"] = work.tile([P, HC, D], BF16, tag=f"Y{hf}", name="Y")
        for il in range(HC):
            pm = ps_mm.tile([P, 512], F32, tag="psmm")
            pr = ps_r.tile([P, 1], F32, tag="psr", name="pr")
            for jc in range(kc):
                nc.tensor.matmul(
                    pm[:],
                    PT[:, jc, il * P:(il + 1) * P],
                    s["VM"][:, jc, :],
                    start=(jc == 0), stop=(jc == kc - 1),
                )
                nc.tensor.matmul(
                    pr[:],
                    PT[:, jc, il * P:(il + 1) * P],
                    ones[:],
                    start=(jc == 0), stop=(jc == kc - 1),
                )
            st = stats.tile([P, 6], F32, tag="bnst")
            nc.vector.bn_stats(st[:], pm[:])
            nc.vector.bn_aggr(MV2[:, b, hf, il, :], st[:])
            nc.scalar.activation(s[f"Y{hf}"][:, il, :], pm[:], ACT.Identity)
            nc.vector.tensor_copy(RALL[:, b, hf, il:il + 1], pr[:])

    def emit_rstd2(b, hf):
        """rstd2 = 1/sqrt(var_u + eps*r^2) via DVE Newton (no scalar sqrt,
        so this runs inside phase A without touching activation tables)."""
        r = RALL[:, b, hf, :]
        sdsq = stats.tile([P, HC], F32, tag="sdsq2")
        nc.vector.scalar_tensor_tensor(
            sdsq[:], r, EPS, r, op0=ALU.mult, op1=ALU.mult
        )
        nc.vector.tensor_tensor(
            sdsq[:], sdsq[:], MV2[:, b, hf, :, 1:2], ALU.add
        )
        newton_rsqrt(RSTD2[:, b, hf, :], sdsq[:], HC)
        nm = stats.tile([P, HC], F32, tag="nm2t")
        nc.vector.tensor_tensor(
            nm[:], MV2[:, b, hf, :, 0:1], RSTD2[:, b, hf, :], ALU.mult
        )
        nc.vector.tensor_scalar(
            NM2[:, b, hf, :], nm[:], -1.0, None, op0=ALU.mult
        )

    def emit_tail_head(b, hf, direct=False):
        """LN2 apply, transpose, fc matmuls -> ZPRES (runs in phase A).
        With direct=True (only legal after gate1), gelu straight from the
        fc psum, skipping the ZPRES round-trip."""
        s = S[b]
        YB = work.tile([P, HC, D], BF16, tag="YB", name="YB")
        for il in range(HC):
            nc.vector.tensor_scalar(
                YB[:, il, :], s[f"Y{hf}"][:, il, :],
                RSTD2[:, b, hf, il:il + 1], NM2[:, b, hf, il:il + 1],
                op0=ALU.mult, op1=ALU.add,
            )
        YLT = work.tile([P, DC, HT_], BF16, tag="YLT", name="YLT")
        for tl in range(HC):
            pt = ps_t.tile([P, DC, P], BF16, tag="pst")
            for c in range(DC):
                nc.tensor.transpose(
                    pt[:, c, :], YB[:, tl, c * P:(c + 1) * P], ident[:]
                )
            nc.vector.tensor_copy(YLT[:, :, tl * P:(tl + 1) * P], pt[:])
        if direct:
            ZT = work.tile([P, DC, HT_], BF16, tag="ZTD", name="ZTD")
            s[f"ZT{hf}"] = ZT
        for c in range(DC):
            pm = ps_mm.tile([P, 512], F32, tag="psmm")
            for ec in range(DC):
                nc.tensor.matmul(
                    pm[:],
                    W["wf"][:, ec, c * P:(c + 1) * P],
                    YLT[:, ec, :],
                    start=(ec == 0), stop=(ec == DC - 1),
                )
            if direct:
                nc.scalar.activation(ZT[:, c, :], pm[:], ACT.Gelu,
                                     scale=gate1[:])
            else:
                nc.scalar.activation(ZPRES[:, b, hf, c, :], pm[:],
                                     ACT.Identity)

    def emit_tail_fin(b, hf, direct=False):
        """gelu (gated to the final gelu-set region), proj, store."""
        if direct:
            ZT = S[b][f"ZT{hf}"]
        else:
            ZT = work.tile([P, DC, HT_], BF16, tag="ZT", name="ZT")
            nc.scalar.activation(
                ZT[:, :, :], ZPRES[:, b, hf, :, :], ACT.Gelu, scale=gate1[:]
            )
        ob = out[b]
        for il in range(HC):
            pm = ps_mm.tile([P, 512], F32, tag="psmm")
            for c in range(DC):
                nc.tensor.matmul(
                    pm[:],
                    ZT[:, c, il * P:(il + 1) * P],
                    W["wp"][:, c, :],
                    start=(c == 0), stop=(c == DC - 1),
                )
            o = outp.tile([P, D], F32, tag="O")
            nc.vector.tensor_copy(o[:], pm[:])
            nc.sync.dma_start(ob[:, hf * HC + il, :], o[:])

    # ---------------- emission ----------------
    # startup DMA order: wv (PE warm-up feed), x[0] (LN1 critical path),
    # then weights interleaved with the remaining batches.
    load_w("wv")
    dma_in(0, split=True)
    load_w("wq")
    dma_in(1)
    load_w("wk")
    dma_in(2)
    load_w("wf")
    dma_in(3)
    load_w("wp")

    def warm_ident(n_mm):
        """HAM warm-up on the identity tile — no DMA dependency."""
        wpm = ps_mm.tile([P, 512], F32, tag="psmm", name="warmmm")
        for k in range(n_mm):
            nc.tensor.matmul(
                wpm[:, 0:P], ident[:], ident[:],
                start=(k == 0), stop=(k == n_mm - 1),
            )

    def warm_burst(k0, n_mm, last):
        wpm = ps_mm.tile([P, 512], F32, tag="psmm", name="warmmm")
        for k in range(n_mm):
            nc.tensor.matmul(
                wpm[:], W["wv"][:, (k0 + k) % DC, 0:P],
                W["wv"][:, (k0 + k) % DC, :],
                start=(k == 0), stop=(k == n_mm - 1),
            )
        if last:
            wsb = outp.tile([P, 8], F32, tag="O", name="warmsb")
            nc.vector.tensor_copy(wsb[:], wpm[:, 0:8])
            nc.sync.dma_start(warm[:], wsb[:])

    warm_ident(30)
    warm_burst(0, 10, False)

    # batch 0 front with fused LN1 stats; stats for the rest (DVE only)
    emit_front(0, kcs[0], with_stats=True)
    warm_burst(16, 8, True)
    for b in range(1, G):
        for t in range(NT):
            ln1_chunk(b, t)
        ln1_finish(b, 0, NT)

    # phase A; each tail_head is delayed half an iteration so its PE work
    # (transposes + fc) never head-of-line-blocks on the LN2 DVE chain.
    # Once the last batch's exps are emitted (end of its D phase), gate1
    # opens and earlier batches' gelu+proj fins are interleaved as PE
    # filler for the final batch's serial E/LN2 chain.
    for b in range(G):
        kc = kcs[b]
        emit_D_alloc(b, 0)
        emit_D_alloc(b, 1)
        for jc in range(kc):
            emit_D(b, 0, jc)
        for jc in range(kc):
            emit_D(b, 1, jc)
        if b >= 1:
            emit_tail_head(b - 1, 1)
        if b == G - 1:
            emit_gate1()
            emit_tail_fin(0, 0)
        emit_E(b, 0, kc)
        emit_rstd2(b, 0)
        if b + 1 < G:
            emit_front(b + 1, kcs[b + 1])
        else:
            emit_tail_fin(0, 1)
            emit_tail_fin(1, 0)
        emit_tail_head(b, 0, direct=(b == G - 1))
        if b == G - 1:
            emit_tail_fin(1, 1)
        emit_E(b, 1, kc)
        emit_rstd2(b, 1)
    emit_tail_fin(2, 0)
    emit_tail_fin(2, 1)
    emit_tail_fin(G - 1, 0, direct=True)
    emit_tail_head(G - 1, 1, direct=True)
    emit_tail_fin(G - 1, 1, direct=True)


def build(kcs):
    from contextlib import ExitStack

    nc = bacc.Bacc("TRN2", target_bir_lowering=False, debug=False,
                   num_devices=NCORES)
    # x/lb/out are pre-rearranged on the host to the on-chip chunk layout
    # [P, NT, ...] (token n = t*P + p) so every DMA line is contiguous.
    x = nc.dram_tensor("x", [G, P, NT, D], BF16, kind="ExternalInput").ap()
    lb = nc.dram_tensor("lb", [G, P, NT], F32, kind="ExternalInput").ap()
    wdram = {
        name: nc.dram_tensor(name, [P, DC, D], BF16, kind="ExternalInput").ap()
        for name in ("wq", "wk", "wv", "wf", "wp")
    }
    out = nc.dram_tensor("out", [G, P, NT, D], F32, kind="ExternalOutput").ap()
    warm = nc.dram_tensor("warm", [P, 8], F32, kind="ExternalOutput").ap()

    with tile.TileContext(nc) as tc:
        with ExitStack() as ctx:
            _body(ctx, tc, x, lb, wdram, out, warm, kcs)
    nc.compile()
    return nc


_NC_CACHE = {}


def get_nc(kcs):
    kcs = tuple(kcs)
    if kcs not in _NC_CACHE:
        _NC_CACHE[kcs] = build(kcs)
    return _NC_CACHE[kcs]


def compute_schedule(belief_base_sizes):
    """Sort batches by live-chunk count desc; slot s takes ranks [8s, 8s+8).

    Returns (order, kcs): order[s*8+c] = original batch index run on core c
    slot s; kcs[s] = max chunk count in slot s (compiled loop bound).
    """
    sizes = np.asarray(belief_base_sizes).astype(np.int64)
    nkc = (sizes + P - 1) // P
    nkc = np.clip(nkc, 1, NT)
    order = np.argsort(-nkc, kind="stable")
    kcs = tuple(int(nkc[order[s * NCORES]]) for s in range(G))
    return order, kcs


def make_in_maps(x, belief_base_sizes, g1, w_qkv, g2, w_fc, w_proj):
    x = np.asarray(x, dtype=np.float32)
    sizes = np.asarray(belief_base_sizes, dtype=np.int64)
    g1 = np.asarray(g1, dtype=np.float32)
    w_qkv = np.asarray(w_qkv, dtype=np.float32)
    g2 = np.asarray(g2, dtype=np.float32)
    w_fc = np.asarray(w_fc, dtype=np.float32)
    w_proj = np.asarray(w_proj, dtype=np.float32)

    bf = ml_dtypes.bfloat16

    def warr(w):
        # device layout [P, DC, D]: partition p holds rows {c*P+p}
        return np.ascontiguousarray(
            w.astype(bf).reshape(DC, P, D).transpose(1, 0, 2))

    wq = warr((g1[:, None] * w_qkv[:, :D]) / np.float32(np.sqrt(D)))
    wk = warr(g1[:, None] * w_qkv[:, D:2 * D])
    wv = warr(g1[:, None] * w_qkv[:, 2 * D:])
    wf = warr(g2[:, None] * w_fc)
    wp = warr(w_proj)

    lbias = np.where(np.arange(N)[None, :] < sizes[:, None],
                     np.float32(0.0), np.float32(NEG)).astype(np.float32)

    order, kcs = compute_schedule(sizes)
    # rearrange to the on-chip chunk layout [P, NT, .] (token n = t*P + p)
    xb = x.astype(bf).reshape(B, NT, P, D).transpose(0, 2, 1, 3)
    lbias = lbias.reshape(B, NT, P).transpose(0, 2, 1)
    in_maps = []
    for c in range(NCORES):
        sel = [int(order[s * NCORES + c]) for s in range(G)]
        in_maps.append({
            "x": np.ascontiguousarray(xb[sel]),
            "lb": np.ascontiguousarray(lbias[sel]),
            "wq": np.ascontiguousarray(wq), "wk": np.ascontiguousarray(wk),
            "wv": np.ascontiguousarray(wv), "wf": np.ascontiguousarray(wf),
            "wp": np.ascontiguousarray(wp),
        })
    return in_maps, order, kcs


def kernel(x, belief_base_sizes, g1, w_qkv, g2, w_fc, w_proj):
    in_maps, order, kcs = make_in_maps(
        x, belief_base_sizes, g1, w_qkv, g2, w_fc, w_proj)
    nc = get_nc(kcs)
    res = run_bass_kernel_spmd(nc, in_maps, core_ids=list(range(NCORES)))
    out = np.empty((B, N, D), dtype=np.float32)
    for c in range(NCORES):
        for s in range(G):
            buf = res.results[c]["out"][s]  # [P, NT, D], token n = t*P + p
            out[int(order[s * NCORES + c])] = (
                buf.transpose(1, 0, 2).reshape(N, D))
    return np.ascontiguousarray(out)


# revision 82
# speedup vs baseline: 1.0150x; 1.0126x over previous
"""Trainium2 Bass kernel for a belief-transformer block (sparse attention).

Computation (per batch b):
    h   = LayerNorm(x[b]) * g1
    qkv = h @ w_qkv ; q,k,v = split(qkv)
    s   = q @ k^T / sqrt(D), keys j >= L_b masked
    y   = softmax(s) @ v
    y   = LayerNorm(y) * g2
    out = gelu(y @ w_fc) @ w_proj

Sharding: data-parallel over batch across 8 NeuronCores (4 slot-batches per
core), weights replicated.  Sparsity: the host sorts batches by
nkc_b = ceil(L_b/128) (number of live 128-key chunks), assigns rank
8s+c to core c slot s, and compiles the program for per-slot chunk
maxima kcs[s] = max over cores.  Key chunks >= kcs[s] are skipped
entirely (k/v production, scores, exp, attn@v); chunks < kcs[s] but
>= the batch's own nkc are masked exactly via the exp bias
(exp(s + log_mask), log_mask in {0, -1e30} per key partition).

Engine layout:
  PE:     transposes (bf16, 1 cyc/col), all matmuls (bf16 operands, fp32
          PSUM, 512-wide moving operands), plus HAM warm-up bursts
  ACT:    exp (log-mask bias), gelu, psum->SBUF Identity copies; the
          activation-table sets are switched only 3x for the whole kernel
          (Sqrt at startup -> Exp for phase A -> Gelu for the fins)
  DVE:    bn_stats/aggr for both LNs, LN applies, Newton-iteration rsqrt
          (quake seed + int ALU ops) so phase A never needs the Sqrt set
  Softmax normalization is absorbed into LN2 (scale invariance), with the
  exact eps correction sd = sqrt(var_u + eps*r^2), r = sum_k exp(s); r
  comes from 1-column matmuls sharing the attn@v stationary operands.

Scheduling: Tile's scheduler reorders each engine's static queue, so
ordering is enforced with data deps, not emission order: gate1 (a 1.0
tile derived from the last batch's exp outputs) gates every Gelu's
scale operand so the single Exp->Gelu table switch happens exactly
once; per-half LN2/fc tails run inside phase A delayed half an
iteration (so the in-order PE queue never head-of-line-blocks on the
LN2 DVE chain), and earlier batches' gelu+proj fins are interleaved
into the final batch's serial chain as PE filler.  x/weights/output
are host-rearranged to the on-chip chunk layout so every DMA line is
4-8KB contiguous.
"""

import os
import sys

import numpy as np

try:
    import ml_dtypes
except ImportError:  # pragma: no cover
    ml_dtypes = None


def _ensure_concourse():
    try:
        import concourse  # noqa: F401
        return
    except ImportError:
        pass
    for p in ("/root/.axon_site/_ro/trn_rl_repo", "/opt/trn_rl_repo"):
        if os.path.isdir(p) and p not in sys.path:
            sys.path.insert(0, p)
    import concourse  # noqa: F401


_ensure_concourse()

import concourse.tile as tile  # noqa: E402
from concourse import bacc, mybir  # noqa: E402
from concourse.bass_utils import run_bass_kernel_spmd  # noqa: E402
from concourse.masks import make_identity  # noqa: E402

B, N, D = 32, 1024, 512
NCORES = 8
G = B // NCORES  # batches (slots) per core
P = 128
NT = N // P      # token chunks (8)
DC = D // P      # feature chunks (4)
HT_ = N // 2     # token half (512)
HC = HT_ // P    # token chunks per half (4)
EPS = 1e-5
NEG = -1e30

F32 = mybir.dt.float32
BF16 = mybir.dt.bfloat16
ALU = mybir.AluOpType
ACT = mybir.ActivationFunctionType


def _body(ctx, tc, x, lb, wdram, out, warm, kcs):
    nc = tc.nc

    singles = ctx.enter_context(tc.tile_pool(name="singles", bufs=1))
    main = ctx.enter_context(tc.tile_pool(name="main", bufs=1))
    vmp = ctx.enter_context(tc.tile_pool(name="vmp", bufs=2))
    work = ctx.enter_context(tc.tile_pool(name="work", bufs=2))
    outp = ctx.enter_context(tc.tile_pool(name="outp", bufs=3))
    stats = ctx.enter_context(tc.tile_pool(name="stats", bufs=2))
    ps_mm = ctx.enter_context(tc.tile_pool(name="ps_mm", bufs=4, space="PSUM"))
    ps_t = ctx.enter_context(tc.tile_pool(name="ps_t", bufs=2, space="PSUM"))
    ps_r = ctx.enter_context(tc.tile_pool(name="ps_r", bufs=2, space="PSUM"))

    # ---- replicated weights, feature-chunked [P, DC, D] bf16 ----
    W = {}

    def load_w(name):
        t = singles.tile([P, DC, D], BF16, tag=name, name=name)
        nc.sync.dma_start(t[:], wdram[name])  # host pre-rearranged layout
        W[name] = t

    ident = singles.tile([P, P], BF16, tag="ident")
    make_identity(nc, ident)
    ones = singles.tile([P, 1], BF16, tag="ones")
    nc.vector.memset(ones[:], 1.0)
    eps_t = singles.tile([P, 1], F32, tag="eps")
    nc.vector.memset(eps_t[:], EPS)


    # persistent per-batch state
    X = singles.tile([P, G, NT, D], BF16, tag="X", name="X")
    LB = singles.tile([P, G, NT], F32, tag="LB", name="LB")
    ZPRES = singles.tile([P, G, 2, DC, HT_], BF16, tag="ZPRES", name="ZPRES")
    MV1 = singles.tile([P, G, NT, 2], F32, tag="MV1", name="MV1")
    RS1 = singles.tile([P, G, NT], F32, tag="RS1", name="RS1")
    NM1 = singles.tile([P, G, NT], F32, tag="NM1", name="NM1")
    MV2 = singles.tile([P, G, 2, HC, 2], F32, tag="MV2", name="MV2")
    RALL = singles.tile([P, G, 2, HC], F32, tag="RALL", name="RALL")
    RSTD2 = singles.tile([P, G, 2, HC], F32, tag="RSTD2", name="RSTD2")
    NM2 = singles.tile([P, G, 2, HC], F32, tag="NM2", name="NM2")

    gate1 = singles.tile([P, 1], F32, tag="gate1", name="gate1")

    S = [dict() for _ in range(G)]  # per-batch transient tiles

    def dma_in(b, split=False):
        # host pre-rearranged x to [P, NT, D]: contiguous 8KB/partition
        if split:
            nc.sync.dma_start(X[:, b, 0:2, :], x[b, :, 0:2, :])
            nc.sync.dma_start(LB[:, b, :], lb[b])
            nc.sync.dma_start(X[:, b, 2:NT, :], x[b, :, 2:NT, :])
        else:
            nc.sync.dma_start(X[:, b, :, :], x[b])
            nc.sync.dma_start(LB[:, b, :], lb[b])

    I32 = mybir.dt.int32

    def newton_rsqrt(dst, vin, w, iters=2):
        """dst[P, w] = 1/sqrt(vin) on DVE (quake seed + Newton iters)."""
        sdsq = stats.tile([P, NT], F32, tag="sdsq", name="sdsq")[:, 0:w]
        nc.vector.tensor_copy(sdsq, vin)
        y = stats.tile([P, NT], F32, tag="nty", name="nty")[:, 0:w]
        nc.vector.tensor_scalar(
            y.bitcast(I32), sdsq.bitcast(I32), 1, -1,
            op0=ALU.logical_shift_right, op1=ALU.bitwise_xor,
        )
        nc.vector.tensor_scalar(
            y.bitcast(I32), y.bitcast(I32), 0x5f3759e0, None, op0=ALU.add,
        )
        for it in range(iters):
            a = stats.tile([P, NT], F32, tag="nta", name="nta")[:, 0:w]
            nc.vector.tensor_tensor(a, y, y, ALU.mult)
            nc.vector.tensor_tensor(a, a, sdsq, ALU.mult)
            nc.vector.tensor_scalar(a, a, -0.5, 1.5, op0=ALU.mult,
                                    op1=ALU.add)
            nc.vector.tensor_tensor(dst if it == iters - 1 else y, y, a,
                                    ALU.mult)

    def ln1_chunk(b, t):
        """LN1 stats for one chunk (DVE only)."""
        st = stats.tile([P, 6], F32, tag="bnst")
        nc.vector.bn_stats(st[:], X[:, b, t, :])
        nc.vector.bn_aggr(MV1[:, b, t, :], st[:])

    def ln1_finish_fast(b, t):
        """Startup-critical rstd1 chain routed through the idle scalar
        engine (Sqrt set loads before the first Exp)."""
        sd = stats.tile([P, 1], F32, tag="sdf")
        nc.scalar.activation(sd[:], MV1[:, b, t, 1:2], ACT.Sqrt,
                             bias=eps_t[:])
        nc.vector.reciprocal(RS1[:, b, t:t + 1], sd[:])
        nc.vector.tensor_scalar(
            NM1[:, b, t:t + 1], MV1[:, b, t, 0:1], RS1[:, b, t:t + 1], -1.0,
            op0=ALU.mult, op1=ALU.mult,
        )

    def ln1_finish(b, t0, t1):
        """rstd1/negmu1 for chunks [t0,t1) via DVE Newton."""
        w = t1 - t0
        veps = stats.tile([P, NT], F32, tag="veps", name="veps")[:, 0:w]
        nc.vector.tensor_scalar(
            veps, MV1[:, b, t0:t1, 1:2], 1.0, EPS, op0=ALU.mult, op1=ALU.add
        )
        newton_rsqrt(RS1[:, b, t0:t1], veps, w, iters=1)
        nm = stats.tile([P, NT], F32, tag="nm1t", name="nm1t")[:, 0:w]
        nc.vector.tensor_tensor(
            nm, MV1[:, b, t0:t1, 0:1], RS1[:, b, t0:t1], ALU.mult
        )
        nc.vector.tensor_scalar(
            NM1[:, b, t0:t1], nm, -1.0, None, op0=ALU.mult
        )

    def emit_gate1():
        """gate1 = 1.0 once the last batch's exp outputs (PT tiles) are
        written — the earliest moment the gelu table switch is legal."""
        s = S[G - 1]
        kc = kcs[G - 1]
        tmp1 = stats.tile([P, 1], F32, tag="gtmp")
        nc.vector.tensor_reduce(tmp1[:], s["PT0"][:, 0:kc, 0:1],
                                axis=mybir.AxisListType.XY, op=ALU.max)
        tmp2 = stats.tile([P, 1], F32, tag="gtmp2")
        nc.vector.tensor_reduce(tmp2[:], s["PT1"][:, 0:kc, 0:1],
                                axis=mybir.AxisListType.XY, op=ALU.max)
        nc.vector.tensor_tensor(tmp1[:], tmp1[:], tmp2[:], ALU.max)
        nc.vector.tensor_scalar(
            gate1[:], tmp1[:], 0.0, 1.0, op0=ALU.mult, op1=ALU.add
        )

    def emit_H_chunk(b, t, on_scalar=False):
        """LN1 apply: H = rstd*x + (-mu*rstd), bf16 in/out."""
        s = S[b]
        if on_scalar:
            nc.scalar.activation(
                s["H"][:, t, :], X[:, b, t, :], ACT.Identity,
                bias=NM1[:, b, t:t + 1], scale=RS1[:, b, t:t + 1],
            )
        else:
            nc.vector.tensor_scalar(
                s["H"][:, t, :], X[:, b, t, :], RS1[:, b, t:t + 1],
                NM1[:, b, t:t + 1], op0=ALU.mult, op1=ALU.add,
            )

    def emit_B_alloc(b):
        s = S[b]
        s["H"] = main.tile([P, NT, D], BF16, tag="H", name="H")
        s["HT"] = main.tile([P, DC, N], BF16, tag="HT", name="HT")
        s["VM"] = vmp.tile([P, NT, D], BF16, tag="VM", name="VM")

    def emit_Bt(b, t, kc):
        """Transpose h chunk t -> h_T; v-matmul for live chunks."""
        s = S[b]
        pt = ps_t.tile([P, DC, P], BF16, tag="pst")
        for c in range(DC):
            nc.tensor.transpose(
                pt[:, c, :], s["H"][:, t, c * P:(c + 1) * P], ident[:]
            )
        nc.vector.tensor_copy(s["HT"][:, :, t * P:(t + 1) * P], pt[:])
        if t < kc:
            pm = ps_mm.tile([P, 512], F32, tag="psmm")
            for dc_ in range(DC):
                nc.tensor.matmul(
                    pm[:],
                    s["HT"][:, dc_, t * P:(t + 1) * P],
                    W["wv"][:, dc_, :],
                    start=(dc_ == 0), stop=(dc_ == DC - 1),
                )
            nc.scalar.activation(s["VM"][:, t, :], pm[:], ACT.Identity)

    def emit_C_alloc(b):
        s = S[b]
        s["QT"] = main.tile([P, DC, N], BF16, tag="QT", name="QT")
        s["KT"] = main.tile([P, DC, N], BF16, tag="KT", name="KT")

    def emit_Q_piece(b, h, c):
        """One PSUM group of q_T production (4 matmuls, 512 tokens)."""
        s = S[b]
        pm = ps_mm.tile([P, 512], F32, tag="psmm")
        for dc_ in range(DC):
            nc.tensor.matmul(
                pm[:],
                W["wq"][:, dc_, c * P:(c + 1) * P],
                s["HT"][:, dc_, h * 512:(h + 1) * 512],
                start=(dc_ == 0), stop=(dc_ == DC - 1),
            )
        nc.scalar.activation(
            s["QT"][:, c, h * 512:(h + 1) * 512], pm[:], ACT.Identity
        )

    def emit_K_span(b, h, c):
        """k_T production for a full 512-key span."""
        s = S[b]
        pm = ps_mm.tile([P, 512], F32, tag="psmm")
        for dc_ in range(DC):
            nc.tensor.matmul(
                pm[:],
                W["wk"][:, dc_, c * P:(c + 1) * P],
                s["HT"][:, dc_, h * 512:(h + 1) * 512],
                start=(dc_ == 0), stop=(dc_ == DC - 1),
            )
        nc.vector.tensor_copy(s["KT"][:, c, h * 512:(h + 1) * 512], pm[:])

    def emit_K_chunk(b, jc, c):
        """k_T production for one 128-key chunk (remainder)."""
        s = S[b]
        pk = ps_mm.tile([P, 512], F32, tag="psmm", name="psk")
        for dc_ in range(DC):
            nc.tensor.matmul(
                pk[:, 0:P],
                W["wk"][:, dc_, c * P:(c + 1) * P],
                s["HT"][:, dc_, jc * P:(jc + 1) * P],
                start=(dc_ == 0), stop=(dc_ == DC - 1),
            )
        nc.vector.tensor_copy(s["KT"][:, c, jc * P:(jc + 1) * P], pk[:, 0:P])

    def emit_front(b, kc, with_stats=False):
        """H, transposes+v, q, k for batch b, pipelined per token chunk."""
        emit_B_alloc(b)
        emit_C_alloc(b)
        nspan = kc // 4

        def chunk_work(t, h_on_scalar=False):
            emit_H_chunk(b, t, on_scalar=h_on_scalar)
            emit_Bt(b, t, kc)
            if t >= 4 * nspan and t < kc:
                for c in range(DC):
                    emit_K_chunk(b, t, c)
            if t == 3:
                for c in range(DC):
                    emit_Q_piece(b, 0, c)
                if nspan >= 1:
                    for c in range(DC):
                        emit_K_span(b, 0, c)
            if t == 7:
                for c in range(DC):
                    emit_Q_piece(b, 1, c)
                if nspan >= 2:
                    for c in range(DC):
                        emit_K_span(b, 1, c)

        if with_stats:
            # first two chunks: rstd chain via the idle scalar engine so
            # the DVE bn backlog can't stretch the critical path
            for t in (0, 1):
                ln1_chunk(b, t)
                ln1_finish_fast(b, t)
                chunk_work(t, h_on_scalar=True)
            for tp in range(1, NT // 2):
                ln1_chunk(b, 2 * tp)
                ln1_chunk(b, 2 * tp + 1)
                ln1_finish(b, 2 * tp, 2 * tp + 2)
                chunk_work(2 * tp)
                chunk_work(2 * tp + 1)
        else:
            for t in range(NT):
                chunk_work(t)

    def emit_D_alloc(b, hf):
        S[b][f"PT{hf}"] = main.tile([P, NT, HT_], BF16, tag=f"PT{hf}",
                                    name="PT")

    def emit_D(b, hf, jc):
        """Scores for key-chunk jc (keys on partitions) + masked exp."""
        s = S[b]
        q0 = hf * HT_
        pm = ps_mm.tile([P, 512], F32, tag="psmm")
        for dc_ in range(DC):
            nc.tensor.matmul(
                pm[:],
                s["KT"][:, dc_, jc * P:(jc + 1) * P],
                s["QT"][:, dc_, q0:q0 + HT_],
                start=(dc_ == 0), stop=(dc_ == DC - 1),
            )
        nc.scalar.activation(
            s[f"PT{hf}"][:, jc, :], pm[:], ACT.Exp, bias=LB[:, b, jc:jc + 1]
        )

    def emit_E(b, hf, kc):
        """y_unnorm = p^T @ v; rowsums r via 1-col matmuls; LN2 stats."""
        s = S[b]
        PT = s[f"PT{hf}"]
        s[f"Y{hf}"] = work.tile([P, HC, D], BF16, tag=f"Y{hf}", name="Y")
        for il in range(HC):
            pm = ps_mm.tile([P, 512], F32, tag="psmm")
            pr = ps_r.tile([P, 1], F32, tag="psr", name="pr")
            for jc in range(kc):
                nc.tensor.matmul(
                    pm[:],
                    PT[:, jc, il * P:(il + 1) * P],
                    s["VM"][:, jc, :],
                    start=(jc == 0), stop=(jc == kc - 1),
                )
                nc.tensor.matmul(
                    pr[:],
                    PT[:, jc, il * P:(il + 1) * P],
                    ones[:],
                    start=(jc == 0), stop=(jc == kc - 1),
                )
            st = stats.tile([P, 6], F32, tag="bnst")
            nc.vector.bn_stats(st[:], pm[:])
            nc.vector.bn_aggr(MV2[:, b, hf, il, :], st[:])
            nc.scalar.activation(s[f"Y{hf}"][:, il, :], pm[:], ACT.Identity)
            nc.vector.tensor_copy(RALL[:, b, hf, il:il + 1], pr[:])

    def emit_rstd2(b, hf):
        """rstd2 = 1/sqrt(var_u + eps*r^2) via DVE Newton (no scalar sqrt,
        so this runs inside phase A without touching activation tables)."""
        r = RALL[:, b, hf, :]
        sdsq = stats.tile([P, HC], F32, tag="sdsq2")
        nc.vector.scalar_tensor_tensor(
            sdsq[:], r, EPS, r, op0=ALU.mult, op1=ALU.mult
        )
        nc.vector.tensor_tensor(
            sdsq[:], sdsq[:], MV2[:, b, hf, :, 1:2], ALU.add
        )
        newton_rsqrt(RSTD2[:, b, hf, :], sdsq[:], HC)
        nm = stats.tile([P, HC], F32, tag="nm2t")
        nc.vector.tensor_tensor(
            nm[:], MV2[:, b, hf, :, 0:1], RSTD2[:, b, hf, :], ALU.mult
        )
        nc.vector.tensor_scalar(
            NM2[:, b, hf, :], nm[:], -1.0, None, op0=ALU.mult
        )

    def emit_tail_head(b, hf, direct=False):
        """LN2 apply, transpose, fc matmuls -> ZPRES (runs in phase A).
        With direct=True (only legal after gate1), gelu straight from the
        fc psum, skipping the ZPRES round-trip."""
        s = S[b]
        YB = work.tile([P, HC, D], BF16, tag="YB", name="YB")
        for il in range(HC):
            nc.vector.tensor_scalar(
                YB[:, il, :], s[f"Y{hf}"][:, il, :],
                RSTD2[:, b, hf, il:il + 1], NM2[:, b, hf, il:il + 1],
                op0=ALU.mult, op1=ALU.add,
            )
        YLT = work.tile([P, DC, HT_], BF16, tag="YLT", name="YLT")
        for tl in range(HC):
            pt = ps_t.tile([P, DC, P], BF16, tag="pst")
            for c in range(DC):
                nc.tensor.transpose(
                    pt[:, c, :], YB[:, tl, c * P:(c + 1) * P], ident[:]
                )
            nc.vector.tensor_copy(YLT[:, :, tl * P:(tl + 1) * P], pt[:])
        if direct:
            ZT = work.tile([P, DC, HT_], BF16, tag="ZTD", name="ZTD")
            s[f"ZT{hf}"] = ZT
        for c in range(DC):
            pm = ps_mm.tile([P, 512], F32, tag="psmm")
            for ec in range(DC):
                nc.tensor.matmul(
                    pm[:],
                    W["wf"][:, ec, c * P:(c + 1) * P],
                    YLT[:, ec, :],
                    start=(ec == 0), stop=(ec == DC - 1),
                )
            if direct:
                nc.scalar.activation(ZT[:, c, :], pm[:], ACT.Gelu,
                                     scale=gate1[:])
            else:
                nc.scalar.activation(ZPRES[:, b, hf, c, :], pm[:],
                                     ACT.Identity)

    def emit_tail_fin(b, hf, direct=False):
        """gelu (gated to the final gelu-set region), proj, store."""
        if direct:
            ZT = S[b][f"ZT{hf}"]
        else:
            ZT = work.tile([P, DC, HT_], BF16, tag="ZT", name="ZT")
            nc.scalar.activation(
                ZT[:, :, :], ZPRES[:, b, hf, :, :], ACT.Gelu, scale=gate1[:]
            )
        ob = out[b]
        for il in range(HC):
            pm = ps_mm.tile([P, 512], F32, tag="psmm")
            for c in range(DC):
                nc.tensor.matmul(
                    pm[:],
                    ZT[:, c, il * P:(il + 1) * P],
                    W["wp"][:, c, :],
                    start=(c == 0), stop=(c == DC - 1),
                )
            o = outp.tile([P, D], F32, tag="O")
            nc.vector.tensor_copy(o[:], pm[:])
            nc.sync.dma_start(ob[:, hf * HC + il, :], o[:])

    # ---------------- emission ----------------
    # startup DMA order: wv (PE warm-up feed), x[0] (LN1 critical path),
    # then weights interleaved with the remaining batches.
    load_w("wv")
    dma_in(0, split=True)
    load_w("wq")
    dma_in(1)
    load_w("wk")
    dma_in(2)
    load_w("wf")
    dma_in(3)
    load_w("wp")

    def warm_ident(n_mm):
        """HAM warm-up on the identity tile — no DMA dependency."""
        wpm = ps_mm.tile([P, 512], F32, tag="psmm", name="warmmm")
        for k in range(n_mm):
            nc.tensor.matmul(
                wpm[:, 0:P], ident[:], ident[:],
                start=(k == 0), stop=(k == n_mm - 1),
            )

    def warm_burst(k0, n_mm, last):
        wpm = ps_mm.tile([P, 512], F32, tag="psmm", name="warmmm")
        for k in range(n_mm):
            nc.tensor.matmul(
                wpm[:], W["wv"][:, (k0 + k) % DC, 0:P],
                W["wv"][:, (k0 + k) % DC, :],
                start=(k == 0), stop=(k == n_mm - 1),
            )
        if last:
            wsb = outp.tile([P, 8], F32, tag="O", name="warmsb")
            nc.vector.tensor_copy(wsb[:], wpm[:, 0:8])
            nc.sync.dma_start(warm[:], wsb[:])

    warm_ident(30)
    warm_burst(0, 10, False)

    # batch 0 front with fused LN1 stats; stats for the rest (DVE only)
    emit_front(0, kcs[0], with_stats=True)
    warm_burst(16, 8, True)
    for b in range(1, G):
        for t in range(NT):
            ln1_chunk(b, t)
        ln1_finish(b, 0, NT)

    # phase A; each tail_head is delayed half an iteration so its PE work
    # (transposes + fc) never head-of-line-blocks on the LN2 DVE chain.
    # Once the last batch's exps are emitted (end of its D phase), gate1
    # opens and earlier batches' gelu+proj fins are interleaved as PE
    # filler for the final batch's serial E/LN2 chain.
    for b in range(G):
        kc = kcs[b]
        emit_D_alloc(b, 0)
        emit_D_alloc(b, 1)
        for jc in range(kc):
            emit_D(b, 0, jc)
        for jc in range(kc):
            emit_D(b, 1, jc)
        if b >= 1:
            emit_tail_head(b - 1, 1)
        if b == G - 1:
            emit_gate1()
            emit_tail_fin(0, 0)
        emit_E(b, 0, kc)
        emit_rstd2(b, 0)
        if b + 1 < G:
            emit_front(b + 1, kcs[b + 1])
        else:
            emit_tail_fin(0, 1)
            emit_tail_fin(1, 0)
        emit_tail_head(b, 0, direct=(b == G - 1))
        if b == G - 1:
            emit_tail_fin(1, 1)
        emit_E(b, 1, kc)
        emit_rstd2(b, 1)
    emit_tail_fin(2, 0)
    emit_tail_fin(2, 1)
    emit_tail_fin(G - 1, 0, direct=True)
    emit_tail_head(G - 1, 1, direct=True)
    emit_tail_fin(G - 1, 1, direct=True)


def build(kcs):
    from contextlib import ExitStack

    nc = bacc.Bacc("TRN2", target_bir_lowering=False, debug=False,
                   num_devices=NCORES)
    # x/lb/out are pre-rearranged on the host to the on-chip chunk layout
    # [P, NT, ...] (token n = t*P + p) so every DMA line is contiguous.
    x = nc.dram_tensor("x", [G, P, NT, D], BF16, kind="ExternalInput").ap()
    lb = nc.dram_tensor("lb", [G, P, NT], F32, kind="ExternalInput").ap()
    wdram = {
        name: nc.dram_tensor(name, [P, DC, D], BF16, kind="ExternalInput").ap()
        for name in ("wq", "wk", "wv", "wf", "wp")
    }
    out = nc.dram_tensor("out", [G, P, NT, D], F32, kind="ExternalOutput").ap()
    warm = nc.dram_tensor("warm", [P, 8], F32, kind="ExternalOutput").ap()

    with tile.TileContext(nc) as tc:
        with ExitStack() as ctx:
            _body(ctx, tc, x, lb, wdram, out, warm, kcs)
    nc.compile()
    return nc


_NC_CACHE = {}


def get_nc(kcs):
    kcs = tuple(kcs)
    if kcs not in _NC_CACHE:
        _NC_CACHE[kcs] = build(kcs)
    return _NC_CACHE[kcs]


def compute_schedule(belief_base_sizes):
    """Sort batches by live-chunk count desc; slot s takes ranks [8s, 8s+8).

    Returns (order, kcs): order[s*8+c] = original batch index run on core c
    slot s; kcs[s] = max chunk count in slot s (compiled loop bound).
    """
    sizes = np.asarray(belief_base_sizes).astype(np.int64)
    nkc = (sizes + P - 1) // P
    nkc = np.clip(nkc, 1, NT)
    order = np.argsort(-nkc, kind="stable")
    kcs = tuple(int(nkc[order[s * NCORES]]) for s in range(G))
    return order, kcs


def make_in_maps(x, belief_base_sizes, g1, w_qkv, g2, w_fc, w_proj):
    x = np.asarray(x, dtype=np.float32)
    sizes = np.asarray(belief_base_sizes, dtype=np.int64)
    g1 = np.asarray(g1, dtype=np.float32)
    w_qkv = np.asarray(w_qkv, dtype=np.float32)
    g2 = np.asarray(g2, dtype=np.float32)
    w_fc = np.asarray(w_fc, dtype=np.float32)
    w_proj = np.asarray(w_proj, dtype=np.float32)

    bf = ml_dtypes.bfloat16

    def warr(w):
        # device layout [P, DC, D]: partition p holds rows {c*P+p}
        return np.ascontiguousarray(
            w.astype(bf).reshape(DC, P, D).transpose(1, 0, 2))

    wq = warr((g1[:, None] * w_qkv[:, :D]) / np.float32(np.sqrt(D)))
    wk = warr(g1[:, None] * w_qkv[:, D:2 * D])
    wv = warr(g1[:, None] * w_qkv[:, 2 * D:])
    wf = warr(g2[:, None] * w_fc)
    wp = warr(w_proj)

    lbias = np.where(np.arange(N)[None, :] < sizes[:, None],
                     np.float32(0.0), np.float32(NEG)).astype(np.float32)

    order, kcs = compute_schedule(sizes)
    # rearrange to the on-chip chunk layout [P, NT, .] (token n = t*P + p)
    xb = x.astype(bf).reshape(B, NT, P, D).transpose(0, 2, 1, 3)
    lbias = lbias.reshape(B, NT, P).transpose(0, 2, 1)
    in_maps = []
    for c in range(NCORES):
        sel = [int(order[s * NCORES + c]) for s in range(G)]
        in_maps.append({
            "x": np.ascontiguousarray(xb[sel]),
            "lb": np.ascontiguousarray(lbias[sel]),
            "wq": np.ascontiguousarray(wq), "wk": np.ascontiguousarray(wk),
            "wv": np.ascontiguousarray(wv), "wf": np.ascontiguousarray(wf),
            "wp": np.ascontiguousarray(wp),
        })
    return in_maps, order, kcs


def kernel(x, belief_base_sizes, g1, w_qkv, g2, w_fc, w_proj):
    in_maps, order, kcs = make_in_maps(
        x, belief_base_sizes, g1, w_qkv, g2, w_fc, w_proj)
    nc = get_nc(kcs)
    res = run_bass_kernel_spmd(nc, in_maps, core_ids=list(range(NCORES)))
    out = np.empty((B, N, D), dtype=np.float32)
    for c in range(NCORES):
        for s in range(G):
            buf = res.results[c]["out"][s]  # [P, NT, D], token n = t*P + p
            out[int(order[s * NCORES + c])] = (
                buf.transpose(1, 0, 2).reshape(N, D))
    return np.ascontiguousarray(out)
